# revision 1
# baseline (speedup 1.0000x reference)
"""GQA int8-KV-cache decode attention on 8 NeuronCores (Bass/Tile).

Sharding: kv-head axis (8 kv heads -> 1 per core), per the tensor-parallel hint.
Host does only tiny prep (RoPE of the single new token, its int8 quantization,
patching it into the per-core cache shards, small-tensor layout transposes).
Device does the full masked attention sweep over the int32 KV cache.

Device per-b pipeline (b = batch, chunk = 128 cache positions):
  HBM --SWDGE cast DMA--> K,V bf16 [128p(s), c, 128(d)] in SBUF (ints exact)
  PE transpose chunk (bf16) -> KT [d, s] PSUM -> DVE copy -> SBUF
  PE matmul scoresT[s,4] = KT.T @ qT  (contraction over d), f32 PSUM
  ACT exp(scoresT * k_scaler[s]/sqrt(D) + mask_bias[s]) -> f32 SBUF
  DVE pexp = exp * v_scaler[s] -> bf16
  PE PV accum over chunks: out[4,129] += pexp.T @ [V_chunk | 1/v_scaler]
    (col 128 recovers the softmax denominator: sum exp*vs*(1/vs) = sum exp)
  epilogue: out[:,0:128] * recip(out[:,128]) -> SBUF, one DMA out per core.

Chunk counts per b are baked from input_pos at build time (positions beyond
input_pos are masked out in the reference, so we never read them).
"""

import os

os.environ.setdefault("JAX_PLATFORMS", "cpu")

import math
import numpy as np

B, H, KVH, D, CACHE = 16, 32, 8, 128, 4096
NREP = H // KVH
NCORES = 8
CHUNK = 128
DV = D + 4  # V supertile row: 128 V cols + recip-vs col + pad

USE_CAST_DMA = os.environ.get("KERNEL_CAST_DMA", "1") == "1"
ITERS = int(os.environ.get("KERNEL_ITERS", "1"))
WRAP = int(os.environ.get("KERNEL_WRAP", "1"))
MODE = os.environ.get("KERNEL_MODE", "full")

_BUILD_CACHE = {}
LAST_RESULTS = None


def _rope(x, cos, sin):
    # x: [B, 1, Hx, D]; cos/sin: [B, 1, D//2]
    c = cos[:, :, None, :]
    s = sin[:, :, None, :]
    xe, xo = x[..., ::2], x[..., 1::2]
    re = xe * c - xo * s
    im = xe * s + xo * c
    return np.stack([re, im], axis=-1).reshape(x.shape).astype(np.float32)


def _build_program(ncs):
    """ncs: tuple of per-batch chunk counts (same for every core)."""
    from contextlib import ExitStack

    import concourse.bacc as bacc
    import concourse.tile as tile
    from concourse import mybir

    nc = bacc.Bacc()
    f32 = mybir.dt.float32
    bf16 = mybir.dt.bfloat16
    i32 = mybir.dt.int32

    ck = nc.dram_tensor("ck", [B, CACHE, D], i32, kind="ExternalInput")
    cv = nc.dram_tensor("cv", [B, CACHE, D], i32, kind="ExternalInput")
    ksc = nc.dram_tensor("ksc", [CHUNK, B, CACHE // CHUNK], f32, kind="ExternalInput")
    vsc = nc.dram_tensor("vsc", [CHUNK, B, CACHE // CHUNK], f32, kind="ExternalInput")
    rvs = nc.dram_tensor("rvs", [CHUNK, B, CACHE // CHUNK], f32, kind="ExternalInput")
    qt = nc.dram_tensor("qt", [CHUNK, B, NREP], bf16, kind="ExternalInput")
    mb = nc.dram_tensor("mb", [CHUNK, B], f32, kind="ExternalInput")
    ident = nc.dram_tensor("ident", [CHUNK, CHUNK], bf16, kind="ExternalInput")
    o = nc.dram_tensor("o", [B, NREP, D], f32, kind="ExternalOutput")

    with tile.TileContext(nc) as tc:
        with ExitStack() as ctx:
            PB = int(os.environ.get("KERNEL_PB", "3"))
            SB = int(os.environ.get("KERNEL_SB", "4"))
            MB = int(os.environ.get("KERNEL_MB", "6"))
            OB = int(os.environ.get("KERNEL_OB", "2"))
            singles = ctx.enter_context(tc.tile_pool(name="singles", bufs=1))
            sup = ctx.enter_context(tc.tile_pool(name="sup", bufs=2))
            ktp_pool = ctx.enter_context(tc.tile_pool(name="ktp", bufs=PB, space="PSUM"))
            sc_pool = ctx.enter_context(tc.tile_pool(name="sc", bufs=PB, space="PSUM"))
            ov_pool = ctx.enter_context(tc.tile_pool(name="ov", bufs=OB, space="PSUM"))
            kt_pool = ctx.enter_context(tc.tile_pool(name="kt", bufs=SB))
            sm_pool = ctx.enter_context(tc.tile_pool(name="sm", bufs=MB))

            out_acc = singles.tile([NREP, B, D], f32)
            if MODE == "dmaonly":
                nc.vector.memset(out_acc, 0.0)

            def body():
                idt = singles.tile([CHUNK, CHUNK], bf16, tag="idt")
                nc.sync.dma_start(out=idt, in_=ident[:, :])
                qt_all = singles.tile([CHUNK, B, NREP], bf16, tag="qta")
                nc.sync.dma_start(out=qt_all, in_=qt[:, :, :])
                ksc_all = singles.tile([CHUNK, B, CACHE // CHUNK], f32, tag="ksa")
                nc.sync.dma_start(out=ksc_all, in_=ksc[:, :, :])
                vsc_all = singles.tile([CHUNK, B, CACHE // CHUNK], f32, tag="vsa")
                nc.sync.dma_start(out=vsc_all, in_=vsc[:, :, :])
                rvs_all = singles.tile([CHUNK, B, CACHE // CHUNK], f32, tag="rva")
                nc.sync.dma_start(out=rvs_all, in_=rvs[:, :, :])
                mb_all = singles.tile([CHUNK, B], f32, tag="mba")
                nc.sync.dma_start(out=mb_all, in_=mb[:, :])

                for b in range(B):
                    nch = ncs[b]
                    ovp = None
                    for it in range(ITERS):
                        ksrc = ck[b, 0 : nch * CHUNK, :].rearrange(
                            "(c p) d -> p c d", p=CHUNK
                        )
                        vsrc = cv[b, 0 : nch * CHUNK, :].rearrange(
                            "(c p) d -> p c d", p=CHUNK
                        )
                        ksup = sup.tile([CHUNK, nch, D], bf16, tag="ksup")
                        vsup = sup.tile([CHUNK, nch, DV], bf16, tag="vsup")
                        if MODE == "computeonly":
                            if it == 0:
                                nc.vector.memset(ksup, 1.0)
                                nc.vector.memset(vsup, 1.0)
                        else:
                            nc.gpsimd.dma_start(out=ksup, in_=ksrc)
                            nc.gpsimd.dma_start(out=vsup[:, :, 0:D], in_=vsrc)
                            # denominator column: 1/v_scaler per position
                            nc.vector.tensor_copy(
                                vsup[:, :, D : D + 1], rvs_all[:, b, 0:nch]
                            )

                        if it == 0:
                            ovp = ov_pool.tile([NREP, D + 1], f32)

                        for c in range(nch if MODE != "dmaonly" else 0):
                            ktp = ktp_pool.tile([CHUNK, CHUNK], bf16)
                            nc.tensor.transpose(ktp, ksup[:, c, :], idt)
                            kt = kt_pool.tile([CHUNK, CHUNK], bf16)
                            nc.vector.tensor_copy(kt, ktp)

                            scp = sc_pool.tile([CHUNK, NREP], f32)
                            nc.tensor.matmul(
                                scp,
                                lhsT=kt,
                                rhs=qt_all[:, b, :],
                                start=True,
                                stop=True,
                            )

                            exp_t = sm_pool.tile([CHUNK, NREP], f32, tag="exp")
                            bias = mb_all[:, b : b + 1] if c == nch - 1 else 0.0
                            nc.scalar.activation(
                                exp_t,
                                scp,
                                mybir.ActivationFunctionType.Exp,
                                bias=bias,
                                scale=ksc_all[:, b, c : c + 1],
                            )
                            pexp = sm_pool.tile([CHUNK, NREP], bf16, tag="pexp")
                            nc.vector.tensor_scalar_mul(
                                pexp, exp_t, vsc_all[:, b, c : c + 1]
                            )

                            nc.tensor.matmul(
                                ovp,
                                lhsT=pexp,
                                rhs=vsup[:, c, 0 : D + 1],
                                start=(c == 0 and it == 0),
                                stop=(c == nch - 1 and it == ITERS - 1),
                            )

                    if MODE != "dmaonly":
                        rec = sm_pool.tile([NREP, 1], f32, tag="rec")
                        nc.vector.reciprocal(rec, ovp[:, D : D + 1])
                        nc.vector.tensor_scalar_mul(
                            out_acc[:, b, :], ovp[:, 0:D], rec[:, 0:1]
                        )

            if WRAP > 1:
                with tc.For_i(0, WRAP, 1):
                    body()
            else:
                body()

            nc.sync.dma_start(
                out=o[:, :, :].rearrange("b r d -> r b d"), in_=out_acc
            )

    nc.compile()
    return nc


def kernel(
    xq,
    xk,
    xv,
    freqs_cos,
    freqs_sin,
    k_scaler,
    v_scaler,
    cache_k,
    cache_v,
    input_pos,
):
    global LAST_RESULTS
    import ml_dtypes
    from concourse.bass_utils import run_bass_kernel_spmd

    bf16 = ml_dtypes.bfloat16
    xq = np.asarray(xq, np.float32)
    xk = np.asarray(xk, np.float32)
    xv = np.asarray(xv, np.float32)
    freqs_cos = np.asarray(freqs_cos, np.float32)
    freqs_sin = np.asarray(freqs_sin, np.float32)
    k_scaler = np.asarray(k_scaler, np.float32)
    v_scaler = np.asarray(v_scaler, np.float32)
    cache_k = np.asarray(cache_k)
    cache_v = np.asarray(cache_v)
    input_pos = np.asarray(input_pos)
    pos = input_pos.astype(np.int64)

    # --- tiny host prep: RoPE + int8 quantization of the single new token ---
    q = _rope(xq, freqs_cos, freqs_sin)[:, 0]  # [B, H, D]
    k = _rope(xk, freqs_cos, freqs_sin)[:, 0]  # [B, KVH, D]
    v_new = xv[:, 0]  # [B, KVH, D]
    k_s = (np.max(np.abs(k), axis=-1, keepdims=True) / np.float32(127.0)).astype(
        np.float32
    ) + np.float32(1e-8)
    v_s = (np.max(np.abs(v_new), axis=-1, keepdims=True) / np.float32(127.0)).astype(
        np.float32
    ) + np.float32(1e-8)
    k_q = np.clip(np.round(k / k_s), -127, 127).astype(cache_k.dtype)
    v_q = np.clip(np.round(v_new / v_s), -127, 127).astype(cache_v.dtype)

    ncs = tuple(int(p) // CHUNK + 1 for p in pos)

    key = (ncs, USE_CAST_DMA, ITERS, WRAP, MODE)
    if key not in _BUILD_CACHE:
        _BUILD_CACHE[key] = _build_program(ncs)
    nc = _BUILD_CACHE[key]

    bidx = np.arange(B)
    identity = np.eye(CHUNK, dtype=np.float32).astype(bf16)
    inv_sqrt_d = np.float32(1.0 / math.sqrt(D))

    # boundary-chunk mask bias [128, B]
    mbias = np.where(
        np.arange(CHUNK, dtype=np.int64)[:, None] <= (pos % CHUNK)[None, :],
        np.float32(0.0),
        np.float32(-1e30),
    ).astype(np.float32)

    def chunk_layout(a):  # [B, CACHE] -> [128, B, 32] with s = c*128 + p
        return np.ascontiguousarray(
            a.reshape(B, CACHE // CHUNK, CHUNK).transpose(2, 0, 1)
        )

    in_maps = []
    for m in range(NCORES):
        ck_m = np.ascontiguousarray(cache_k[:, m])  # [B, CACHE, D] i32
        cv_m = np.ascontiguousarray(cache_v[:, m])
        ck_m[bidx, pos, :] = k_q[:, m]
        cv_m[bidx, pos, :] = v_q[:, m]

        ks_m = k_scaler[:, m].copy()  # [B, CACHE]
        vs_m = v_scaler[:, m].copy()
        ks_m[bidx, pos] = k_s[:, m, 0]
        vs_m[bidx, pos] = v_s[:, m, 0]
        ks_m *= inv_sqrt_d

        qt_m = np.ascontiguousarray(
            q[:, m * NREP : (m + 1) * NREP, :].transpose(2, 0, 1)
        ).astype(bf16)  # [D, B, NREP]

        in_maps.append(
            dict(
                ck=ck_m,
                cv=cv_m,
                ksc=chunk_layout(ks_m),
                vsc=chunk_layout(vs_m),
                rvs=chunk_layout(np.float32(1.0) / vs_m),
                qt=qt_m,
                mb=np.ascontiguousarray(mbias),
                ident=identity.copy(),
            )
        )

    res = run_bass_kernel_spmd(nc, in_maps, core_ids=list(range(NCORES)))
    LAST_RESULTS = res

    out = np.zeros((B, H, 1, D), np.float32)
    for m in range(NCORES):
        out[:, m * NREP : (m + 1) * NREP, 0, :] = res.results[m]["o"]
    return out



# revision 7
# speedup vs baseline: 1.6920x; 1.6920x over previous
"""GQA int8-KV-cache decode attention on 8 NeuronCores (Bass/Tile), v2.

Sharding: kv-head axis (8 kv heads -> 1 per core), tensor parallel.
Host does tiny prep (RoPE + int8 quant of the new token, cache patch,
layout shuffles); device does the masked attention sweep.

Key layout/algorithm choices vs the naive version:
  * K/V caches shipped to HBM as int8 (values are -127..127), cast to
    bf16 by the SWDGE DMA on the way into SBUF: 4x less HBM traffic.
  * K stored pre-transposed [B, D, CACHE] so each chunk arrives as
    KT [d, s] and feeds matmul directly -- no PE transpose, no PSUM
    round trip, no DVE copy.
  * V stored position-interleaved [B, CACHE/512, 128, 4, D] so SBUF
    partition p holds positions {g*512 + j*128 + p}: per-partition
    contiguous runs are 512B (full DMA bandwidth) while the SBUF tile
    still has partition = position-within-chunk.
  * Batches sorted by chunk count and DMA'd in groups of 4 (padded to
    the group max) to amortize the ~1us SWDGE descriptor-gen cost.
  * Mask handled entirely via scalers: for positions > input_pos the
    host zeroes k_scaler & 1/v_scaler and sets ln(v_scaler) = -1e30,
    so exp() emits exactly 0 there. No mask bias tensor on device.
  * v_scaler multiply folded into the exp bias: pexp = exp(s*ksc+ln vs).
  * softmax denominator accumulated via a 1-column matmul against
    1/v_scaler; numerator [d, r] and denominator [1, r] are shipped out
    unnormalized and divided on the host.
  * scores->exp->PV chain software-pipelined (PV lags scores by LAG
    chunks) so the PE never head-of-line blocks on the ACT engine.

Per chunk (128 cache positions) the device does:
  PE:  scp[s,r]  = KT_chunk.T @ qT          (contract d, 4-col out)
  ACT: pexp[s,r] = exp(scp*ksc[s] + lnvs[s])  -> bf16
  PE:  ov[d,r]  += V_chunk.T @ pexp          (contract s)
  PE:  den[1,r] += rvs_chunk.T @ pexp        (contract s)
"""

import os

os.environ.setdefault("JAX_PLATFORMS", "cpu")

import math
import numpy as np

B, H, KVH, D, CACHE = 16, 32, 8, 128, 4096
NREP = H // KVH
NCORES = 8
CHUNK = 128
VG = 4  # V chunks interleaved per 512B-run group
NBG = 4  # batches per DMA group

LAG = int(os.environ.get("KERNEL_LAG", "3"))

_BUILD_CACHE = {}
LAST_RESULTS = None


def _rope(x, cos, sin):
    # x: [B, 1, Hx, D]; cos/sin: [B, 1, D//2]
    c = cos[:, :, None, :]
    s = sin[:, :, None, :]
    xe, xo = x[..., ::2], x[..., 1::2]
    re = xe * c - xo * s
    im = xe * s + xo * c
    return np.stack([re, im], axis=-1).reshape(x.shape).astype(np.float32)


def _build_program(ncs):
    """ncs: per-device-batch chunk counts, sorted ascending (same for
    every core since sharding is by kv head)."""
    from contextlib import ExitStack

    import concourse.bacc as bacc
    import concourse.tile as tile
    from concourse import mybir

    nc = bacc.Bacc()
    f32 = mybir.dt.float32
    bf16 = mybir.dt.bfloat16
    i8 = mybir.dt.int8

    NCHG = [
        -(-max(ncs[g * NBG : (g + 1) * NBG]) // VG) * VG for g in range(B // NBG)
    ]

    kt8 = nc.dram_tensor("kt8", [B, D, CACHE], i8, kind="ExternalInput")
    # one V tensor per DMA group, [ngg, NBG, 128, VG, D] so (g, b) merge
    # into a single AP dim and the cast DMA balances within 3 dims
    v8g = [
        nc.dram_tensor(
            f"v8g{g}", [NCHG[g] // VG, NBG, CHUNK, VG, D], i8, kind="ExternalInput"
        )
        for g in range(B // NBG)
    ]
    qt = nc.dram_tensor("qt", [CHUNK, B, NREP], bf16, kind="ExternalInput")
    ksc = nc.dram_tensor("ksc", [CHUNK, B, CACHE // CHUNK], f32, kind="ExternalInput")
    lnv = nc.dram_tensor("lnv", [CHUNK, B, CACHE // CHUNK], f32, kind="ExternalInput")
    rvs = nc.dram_tensor("rvs", [CHUNK, B, CACHE // CHUNK], bf16, kind="ExternalInput")
    o_num = nc.dram_tensor("o_num", [CHUNK, B, NREP], f32, kind="ExternalOutput")
    o_den = nc.dram_tensor("o_den", [1, B, NREP], f32, kind="ExternalOutput")

    with tile.TileContext(nc) as tc:
        with ExitStack() as ctx:
            singles = ctx.enter_context(tc.tile_pool(name="singles", bufs=1))
            sup = ctx.enter_context(tc.tile_pool(name="sup", bufs=2))
            sc_pool = ctx.enter_context(tc.tile_pool(name="sc", bufs=5, space="PSUM"))
            ov_pool = ctx.enter_context(tc.tile_pool(name="ov", bufs=2, space="PSUM"))
            dn_pool = ctx.enter_context(tc.tile_pool(name="dn", bufs=1, space="PSUM"))
            sm_pool = ctx.enter_context(tc.tile_pool(name="sm", bufs=LAG + 3))

            qt_all = singles.tile([CHUNK, B, NREP], bf16, tag="qta")
            nc.sync.dma_start(out=qt_all, in_=qt[:, :, :])
            ksc_all = singles.tile([CHUNK, B, CACHE // CHUNK], f32, tag="ksa")
            nc.sync.dma_start(out=ksc_all, in_=ksc[:, :, :])
            lnv_all = singles.tile([CHUNK, B, CACHE // CHUNK], f32, tag="lva")
            nc.sync.dma_start(out=lnv_all, in_=lnv[:, :, :])
            rvs_all = singles.tile([CHUNK, B, CACHE // CHUNK], bf16, tag="rva")
            nc.sync.dma_start(out=rvs_all, in_=rvs[:, :, :])
            out_num = singles.tile([CHUNK, B, NREP], f32, tag="onum")
            den_acc = singles.tile([1, B, NREP], f32, tag="oden")

            for gi in range(B // NBG):
                b0 = gi * NBG
                nchg = NCHG[gi]
                ksrc = kt8[b0 : b0 + NBG, :, 0 : nchg * CHUNK].rearrange(
                    "b d (c s) -> d b c s", s=CHUNK
                )
                ktsup = sup.tile([CHUNK, NBG, nchg, CHUNK], bf16, tag="ksup")
                nc.gpsimd.dma_start(out=ktsup, in_=ksrc)
                vsrc = v8g[gi][:, :, :, :, :].rearrange("g b p j d -> p g b j d")
                vsup = sup.tile(
                    [CHUNK, nchg // VG, NBG, VG, CHUNK], bf16, tag="vsup"
                )
                nc.gpsimd.dma_start(out=vsup, in_=vsrc)

                for bi in range(NBG):
                    b = b0 + bi
                    nch = ncs[b]
                    ov = ov_pool.tile([CHUNK, NREP], f32)
                    den = dn_pool.tile([1, NREP], f32)
                    pexps = [None] * nch
                    for i in range(nch + LAG):
                        if i < nch:
                            scp = sc_pool.tile([CHUNK, NREP], f32)
                            nc.tensor.matmul(
                                scp,
                                lhsT=ktsup[:, bi, i, :],
                                rhs=qt_all[:, b, :],
                                start=True,
                                stop=True,
                            )
                            px = sm_pool.tile([CHUNK, NREP], bf16, tag="pexp")
                            nc.scalar.activation(
                                px,
                                scp,
                                mybir.ActivationFunctionType.Exp,
                                bias=lnv_all[:, b, i : i + 1],
                                scale=ksc_all[:, b, i : i + 1],
                            )
                            pexps[i] = px
                        j = i - LAG
                        if j >= 0:
                            px = pexps[j]
                            nc.tensor.matmul(
                                ov,
                                lhsT=vsup[:, j // VG, bi, j % VG, :],
                                rhs=px,
                                start=(j == 0),
                                stop=(j == nch - 1),
                            )
                            nc.tensor.matmul(
                                den,
                                lhsT=rvs_all[:, b, j : j + 1],
                                rhs=px,
                                start=(j == 0),
                                stop=(j == nch - 1),
                            )
                            pexps[j] = None

                    nc.vector.tensor_copy(out_num[:, b, :], ov)
                    nc.vector.tensor_copy(den_acc[:, b, :], den)

            nc.sync.dma_start(out=o_num[:, :, :], in_=out_num)
            nc.sync.dma_start(out=o_den[:, :, :], in_=den_acc)

    nc.compile()
    return nc


def kernel(
    xq,
    xk,
    xv,
    freqs_cos,
    freqs_sin,
    k_scaler,
    v_scaler,
    cache_k,
    cache_v,
    input_pos,
):
    global LAST_RESULTS
    import ml_dtypes
    from concourse.bass_utils import run_bass_kernel_spmd

    bf16 = ml_dtypes.bfloat16
    xq = np.asarray(xq, np.float32)
    xk = np.asarray(xk, np.float32)
    xv = np.asarray(xv, np.float32)
    freqs_cos = np.asarray(freqs_cos, np.float32)
    freqs_sin = np.asarray(freqs_sin, np.float32)
    k_scaler = np.asarray(k_scaler, np.float32)
    v_scaler = np.asarray(v_scaler, np.float32)
    cache_k = np.asarray(cache_k)
    cache_v = np.asarray(cache_v)
    input_pos = np.asarray(input_pos)
    pos = input_pos.astype(np.int64)

    # --- tiny host prep: RoPE + int8 quantization of the single new token ---
    q = _rope(xq, freqs_cos, freqs_sin)[:, 0]  # [B, H, D]
    k = _rope(xk, freqs_cos, freqs_sin)[:, 0]  # [B, KVH, D]
    v_new = xv[:, 0]  # [B, KVH, D]
    k_s = (np.max(np.abs(k), axis=-1, keepdims=True) / np.float32(127.0)).astype(
        np.float32
    ) + np.float32(1e-8)
    v_s = (np.max(np.abs(v_new), axis=-1, keepdims=True) / np.float32(127.0)).astype(
        np.float32
    ) + np.float32(1e-8)
    k_q = np.clip(np.round(k / k_s), -127, 127).astype(np.int8)
    v_q = np.clip(np.round(v_new / v_s), -127, 127).astype(np.int8)

    # device batch order: ascending chunk count (shrinks DMA-group padding)
    ncs_raw = pos // CHUNK + 1
    order = np.argsort(ncs_raw, kind="stable")
    ncs = tuple(int(ncs_raw[b]) for b in order)

    if ncs not in _BUILD_CACHE:
        _BUILD_CACHE[ncs] = _build_program(ncs)
    nc = _BUILD_CACHE[ncs]

    bidx = np.arange(B)
    inv_sqrt_d = np.float32(1.0 / math.sqrt(D))
    # valid-position mask in chunk layout applied below via scalers
    s_idx = np.arange(CACHE, dtype=np.int64)
    masked = s_idx[None, :] > pos[:, None]  # [B, CACHE] True -> excluded

    def chunk_layout(a):  # [B, CACHE] -> [128, B, 32] with s = c*128 + p
        return np.ascontiguousarray(
            a.reshape(B, CACHE // CHUNK, CHUNK).transpose(2, 0, 1)
        )

    in_maps = []
    for m in range(NCORES):
        ck_m = cache_k[:, m].astype(np.int8)  # [B, CACHE, D]
        cv_m = cache_v[:, m].astype(np.int8)
        ck_m[bidx, pos, :] = k_q[:, m]
        cv_m[bidx, pos, :] = v_q[:, m]
        ck_m = ck_m[order]
        cv_m = cv_m[order]

        kt8 = np.ascontiguousarray(ck_m.transpose(0, 2, 1))  # [B, D, CACHE]
        vfull = cv_m.reshape(B, CACHE // (VG * CHUNK), VG, CHUNK, D)
        v8gs = {}
        for g in range(B // NBG):
            b0 = g * NBG
            ngg = -(-max(ncs[b0 : b0 + NBG]) // VG)
            v8gs[f"v8g{g}"] = np.ascontiguousarray(
                vfull[b0 : b0 + NBG, 0:ngg].transpose(1, 0, 3, 2, 4)
            )  # [ngg, NBG, 128, VG, D]

        ks_m = k_scaler[:, m].copy()  # [B, CACHE]
        vs_m = v_scaler[:, m].copy()
        ks_m[bidx, pos] = k_s[:, m, 0]
        vs_m[bidx, pos] = v_s[:, m, 0]
        ks_m = ks_m[order]
        vs_m = vs_m[order]

        ks_m *= inv_sqrt_d
        ks_m[masked[order]] = np.float32(0.0)
        lnv_m = np.log(vs_m).astype(np.float32)
        lnv_m[masked[order]] = np.float32(-1e30)
        rvs_m = (np.float32(1.0) / vs_m).astype(np.float32)
        rvs_m[masked[order]] = np.float32(0.0)

        qt_m = np.ascontiguousarray(
            q[order][:, m * NREP : (m + 1) * NREP, :].transpose(2, 0, 1)
        ).astype(bf16)  # [D, B, NREP]

        in_maps.append(
            dict(
                kt8=kt8,
                **v8gs,
                qt=qt_m,
                ksc=chunk_layout(ks_m),
                lnv=chunk_layout(lnv_m),
                rvs=chunk_layout(rvs_m).astype(bf16),
            )
        )

    res = run_bass_kernel_spmd(nc, in_maps, core_ids=list(range(NCORES)))
    LAST_RESULTS = res

    inv_order = np.empty(B, np.int64)
    inv_order[order] = np.arange(B)
    out = np.zeros((B, H, 1, D), np.float32)
    for m in range(NCORES):
        num = np.asarray(res.results[m]["o_num"], np.float32)  # [D, B, NREP]
        den = np.asarray(res.results[m]["o_den"], np.float32)  # [1, B, NREP]
        o = (num / den).transpose(1, 2, 0)  # [B, NREP, D] (device order)
        out[:, m * NREP : (m + 1) * NREP, 0, :] = o[inv_order]
    return out


# revision 9
# speedup vs baseline: 2.1213x; 1.2537x over previous
"""GQA int8-KV-cache decode attention on 8 NeuronCores (Bass/Tile), v3.

Sharding: kv-head axis (8 kv heads -> 1 per core), tensor parallel.
Host does tiny prep (RoPE + int8 quant of the new token, cache patch,
layout shuffles); device does the masked attention sweep.

Layout/algorithm choices:
  * K/V caches shipped to HBM as bf16 (cast on host). The DMA engines
    charge the max(src,dst) side, so an int8->bf16 cast DMA costs the
    same as a raw bf16 copy -- but raw copies ride the hardware DGE
    (no ~1us/DMA SWDGE desc-gen on GpSimd, no queue DRAIN stalls), so
    exact per-batch loads with no group padding become free.
  * K stored pre-transposed [B, D, CACHE] so each chunk arrives as
    KT [d, s] and feeds matmul directly -- no PE transpose.
  * V stored position-interleaved [B, 8, 128, 4, D] so SBUF partition p
    holds positions {g*512 + j*128 + p}: per-partition contiguous runs
    are 1KB (full DMA bandwidth) while the SBUF tile keeps partition =
    position-within-chunk.
  * One K DMA + one V DMA per batch on the sync-engine HWDGE queue,
    sized exactly to that batch's chunk count.
  * Scores for G=8 chunks accumulate into one PSUM tile; the k_scaler
    multiply is a single DVE tensor_tensor against a stride-0-broadcast
    [128, G, 1->4] AP, then ONE batched ACT exp per G chunks (the ACT
    per-instruction PSUM-access overhead was the v2 bottleneck).
  * Mask via scalers only: masked positions have k_scaler = 0 (so
    exp(0) = 1 exactly) and v_scaler = 0 (no numerator contribution);
    the denominator over-count is the masked-position count, which the
    host subtracts exactly. No mask tensor, no -inf bias.
  * Denominator = sum_s exp accumulated as a per-partition vector on
    DVE (reduce over chunks + add); host does the final 128-partition
    sum. No per-chunk PE matmul for the denominator.
  * Numerator ov[d, r] += V_chunk.T @ pexp accumulates in PSUM across
    chunks; numerator and denominator ship out unnormalized, host
    divides.
  * scores->exp->PV chain software-pipelined at group granularity so
    the PE never head-of-line blocks on ACT/DVE.
"""

import os

os.environ.setdefault("JAX_PLATFORMS", "cpu")

import math
import numpy as np

B, H, KVH, D, CACHE = 16, 32, 8, 128, 4096
NREP = H // KVH
NCORES = 8
CHUNK = 128
VG = 4  # V chunks interleaved per 512B-run group
NBG = 4  # batches per DMA group
G = int(os.environ.get("KERNEL_G", "8"))  # chunks per ACT/exp batch

_BUILD_CACHE = {}
LAST_RESULTS = None


def _rope(x, cos, sin):
    # x: [B, 1, Hx, D]; cos/sin: [B, 1, D//2]
    c = cos[:, :, None, :]
    s = sin[:, :, None, :]
    xe, xo = x[..., ::2], x[..., 1::2]
    re = xe * c - xo * s
    im = xe * s + xo * c
    return np.stack([re, im], axis=-1).reshape(x.shape).astype(np.float32)


def _build_program(ncs):
    """ncs: per-device-batch chunk counts, sorted ascending (same for
    every core since sharding is by kv head)."""
    from contextlib import ExitStack

    import concourse.bacc as bacc
    import concourse.tile as tile
    from concourse import mybir
    from concourse.bass import AP

    nc = bacc.Bacc()
    f32 = mybir.dt.float32
    bf16 = mybir.dt.bfloat16
    mult = mybir.AluOpType.mult
    add = mybir.AluOpType.add

    kt16 = nc.dram_tensor("kt16", [B, D, CACHE], bf16, kind="ExternalInput")
    v16 = nc.dram_tensor(
        "v16", [B, CACHE // (VG * CHUNK), CHUNK, VG, D], bf16, kind="ExternalInput"
    )
    qt = nc.dram_tensor("qt", [CHUNK, B, NREP], bf16, kind="ExternalInput")
    ksc = nc.dram_tensor("ksc", [CHUNK, B, CACHE // CHUNK], f32, kind="ExternalInput")
    vsc = nc.dram_tensor("vsc", [CHUNK, B, CACHE // CHUNK], f32, kind="ExternalInput")
    o_num = nc.dram_tensor("o_num", [CHUNK, B, NREP], f32, kind="ExternalOutput")
    o_den = nc.dram_tensor("o_den", [CHUNK, B, NREP], f32, kind="ExternalOutput")

    def bc(ap, n):
        # append a stride-0 axis of size n (free-axis broadcast)
        return AP(ap.tensor, ap.offset, list(ap.ap) + [[0, n]])

    with tile.TileContext(nc) as tc:
        with ExitStack() as ctx:
            singles = ctx.enter_context(tc.tile_pool(name="singles", bufs=1))
            sup = ctx.enter_context(tc.tile_pool(name="sup", bufs=2))
            sc_pool = ctx.enter_context(tc.tile_pool(name="sc", bufs=3, space="PSUM"))
            ov_pool = ctx.enter_context(tc.tile_pool(name="ov", bufs=2, space="PSUM"))
            sm_pool = ctx.enter_context(tc.tile_pool(name="sm", bufs=3))

            qt_all = singles.tile([CHUNK, B, NREP], bf16, tag="qta")
            nc.sync.dma_start(out=qt_all, in_=qt[:, :, :])
            ksc_all = singles.tile([CHUNK, B, CACHE // CHUNK], f32, tag="ksa")
            nc.sync.dma_start(out=ksc_all, in_=ksc[:, :, :])
            vsc_all = singles.tile([CHUNK, B, CACHE // CHUNK], f32, tag="vsa")
            nc.sync.dma_start(out=vsc_all, in_=vsc[:, :, :])
            out_num = singles.tile([CHUNK, B, NREP], f32, tag="onum")
            den_vec = singles.tile([CHUNK, B, NREP], f32, tag="oden")
            nc.vector.memset(den_vec, 0.0)

            for b in range(B):
                    nch = ncs[b]
                    ngg = -(-nch // VG)
                    ksrc = kt16[b, :, 0 : nch * CHUNK].rearrange(
                        "d (c s) -> d c s", s=CHUNK
                    )
                    ktsup = sup.tile([CHUNK, nch, CHUNK], bf16, tag="ksup")
                    nc.sync.dma_start(out=ktsup, in_=ksrc)
                    vsrc = v16[b, 0:ngg].rearrange("g p j d -> p g j d")
                    vsup = sup.tile([CHUNK, ngg, VG, CHUNK], bf16, tag="vsup")
                    nc.sync.dma_start(out=vsup, in_=vsrc)
                    ngrp = -(-nch // G)
                    grps = [(k * G, min(G, nch - k * G)) for k in range(ngrp)]
                    ov = ov_pool.tile([CHUNK, NREP], f32)
                    pxs = [None] * ngrp

                    def front(k):
                        c0, gsz = grps[k]
                        scb = sc_pool.tile([CHUNK, G, NREP], f32)
                        for g in range(gsz):
                            nc.tensor.matmul(
                                scb[:, g, :],
                                lhsT=ktsup[:, c0 + g, :],
                                rhs=qt_all[:, b, :],
                                start=True,
                                stop=True,
                            )
                        sc_v = scb[:, 0:gsz, :]
                        nc.vector.tensor_tensor(
                            out=sc_v,
                            in0=sc_v,
                            in1=bc(ksc_all[:, b, c0 : c0 + gsz], NREP),
                            op=mult,
                        )
                        px = sm_pool.tile([CHUNK, G, NREP], bf16, tag="px")
                        px_v = px[:, 0:gsz, :]
                        nc.scalar.activation(
                            px_v, sc_v, mybir.ActivationFunctionType.Exp
                        )
                        tmp = sm_pool.tile([CHUNK, NREP], f32, tag="tmp")
                        nc.vector.tensor_reduce(
                            tmp,
                            px_v.rearrange("p g r -> p r g"),
                            axis=mybir.AxisListType.X,
                            op=add,
                        )
                        nc.vector.tensor_tensor(
                            out=den_vec[:, b, :],
                            in0=den_vec[:, b, :],
                            in1=tmp,
                            op=add,
                        )
                        nc.vector.tensor_tensor(
                            out=px_v,
                            in0=px_v,
                            in1=bc(vsc_all[:, b, c0 : c0 + gsz], NREP),
                            op=mult,
                        )
                        pxs[k] = px

                    def back(k):
                        c0, gsz = grps[k]
                        px = pxs[k]
                        for g in range(gsz):
                            c = c0 + g
                            nc.tensor.matmul(
                                ov,
                                lhsT=vsup[:, c // VG, c % VG, :],
                                rhs=px[:, g, :],
                                start=(c == 0),
                                stop=(c == nch - 1),
                            )
                        pxs[k] = None

                    front(0)
                    for k in range(1, ngrp):
                        front(k)
                        back(k - 1)
                    back(ngrp - 1)

                    nc.vector.tensor_copy(out_num[:, b, :], ov)

            nc.sync.dma_start(out=o_num[:, :, :], in_=out_num)
            nc.sync.dma_start(out=o_den[:, :, :], in_=den_vec)

    nc.compile()
    return nc


def kernel(
    xq,
    xk,
    xv,
    freqs_cos,
    freqs_sin,
    k_scaler,
    v_scaler,
    cache_k,
    cache_v,
    input_pos,
):
    global LAST_RESULTS
    import ml_dtypes
    from concourse.bass_utils import run_bass_kernel_spmd

    bf16 = ml_dtypes.bfloat16
    xq = np.asarray(xq, np.float32)
    xk = np.asarray(xk, np.float32)
    xv = np.asarray(xv, np.float32)
    freqs_cos = np.asarray(freqs_cos, np.float32)
    freqs_sin = np.asarray(freqs_sin, np.float32)
    k_scaler = np.asarray(k_scaler, np.float32)
    v_scaler = np.asarray(v_scaler, np.float32)
    cache_k = np.asarray(cache_k)
    cache_v = np.asarray(cache_v)
    input_pos = np.asarray(input_pos)
    pos = input_pos.astype(np.int64)

    # --- tiny host prep: RoPE + int8 quantization of the single new token ---
    q = _rope(xq, freqs_cos, freqs_sin)[:, 0]  # [B, H, D]
    k = _rope(xk, freqs_cos, freqs_sin)[:, 0]  # [B, KVH, D]
    v_new = xv[:, 0]  # [B, KVH, D]
    k_s = (np.max(np.abs(k), axis=-1, keepdims=True) / np.float32(127.0)).astype(
        np.float32
    ) + np.float32(1e-8)
    v_s = (np.max(np.abs(v_new), axis=-1, keepdims=True) / np.float32(127.0)).astype(
        np.float32
    ) + np.float32(1e-8)
    k_q = np.clip(np.round(k / k_s), -127, 127).astype(np.int8)
    v_q = np.clip(np.round(v_new / v_s), -127, 127).astype(np.int8)

    # device batch order: ascending chunk count (shrinks DMA-group padding)
    ncs_raw = pos // CHUNK + 1
    order = np.argsort(ncs_raw, kind="stable")
    ncs = tuple(int(ncs_raw[b]) for b in order)

    if ncs not in _BUILD_CACHE:
        _BUILD_CACHE[ncs] = _build_program(ncs)
    nc = _BUILD_CACHE[ncs]

    bidx = np.arange(B)
    inv_sqrt_d = np.float32(1.0 / math.sqrt(D))
    s_idx = np.arange(CACHE, dtype=np.int64)
    masked = s_idx[None, :] > pos[:, None]  # [B, CACHE] True -> excluded
    masked_dev = masked[order]
    pos_dev = pos[order]
    # masked positions inside processed chunks contribute exp(0)=1 each
    n_masked = (np.asarray(ncs, np.int64) * CHUNK - (pos_dev + 1)).astype(
        np.float32
    )  # [B] device order

    def chunk_layout(a):  # [B, CACHE] -> [128, B, 32] with s = c*128 + p
        return np.ascontiguousarray(
            a.reshape(B, CACHE // CHUNK, CHUNK).transpose(2, 0, 1)
        )

    in_maps = []
    for m in range(NCORES):
        ck_m = cache_k[:, m].astype(np.int8)  # [B, CACHE, D]
        cv_m = cache_v[:, m].astype(np.int8)
        ck_m[bidx, pos, :] = k_q[:, m]
        cv_m[bidx, pos, :] = v_q[:, m]
        ck_m = ck_m[order].astype(bf16)
        cv_m = cv_m[order].astype(bf16)

        kt16 = np.ascontiguousarray(ck_m.transpose(0, 2, 1))  # [B, D, CACHE]
        v16 = np.ascontiguousarray(
            cv_m.reshape(B, CACHE // (VG * CHUNK), VG, CHUNK, D).transpose(
                0, 1, 3, 2, 4
            )
        )  # [B, 8, 128, VG, D]

        ks_m = k_scaler[:, m].copy()  # [B, CACHE]
        vs_m = v_scaler[:, m].copy()
        ks_m[bidx, pos] = k_s[:, m, 0]
        vs_m[bidx, pos] = v_s[:, m, 0]
        ks_m = ks_m[order]
        vs_m = vs_m[order]

        ks_m *= inv_sqrt_d
        ks_m[masked_dev] = np.float32(0.0)
        vs_m[masked_dev] = np.float32(0.0)

        qt_m = np.ascontiguousarray(
            q[order][:, m * NREP : (m + 1) * NREP, :].transpose(2, 0, 1)
        ).astype(bf16)  # [D, B, NREP]

        in_maps.append(
            dict(
                kt16=kt16,
                v16=v16,
                qt=qt_m,
                ksc=chunk_layout(ks_m),
                vsc=chunk_layout(vs_m),
            )
        )

    res = run_bass_kernel_spmd(nc, in_maps, core_ids=list(range(NCORES)))
    LAST_RESULTS = res

    inv_order = np.empty(B, np.int64)
    inv_order[order] = np.arange(B)
    out = np.zeros((B, H, 1, D), np.float32)
    for m in range(NCORES):
        num = np.asarray(res.results[m]["o_num"], np.float32)  # [D, B, NREP]
        dvec = np.asarray(res.results[m]["o_den"], np.float32)  # [128, B, NREP]
        den = dvec.sum(axis=0) - n_masked[:, None]  # [B, NREP]
        o = (num / den[None, :, :]).transpose(1, 2, 0)  # [B, NREP, D]
        out[:, m * NREP : (m + 1) * NREP, 0, :] = o[inv_order]
    return out


# revision 10
# speedup vs baseline: 2.1824x; 1.0288x over previous
"""GQA int8-KV-cache decode attention on 8 NeuronCores (Bass/Tile), v3.

Sharding: kv-head axis (8 kv heads -> 1 per core), tensor parallel.
Host does tiny prep (RoPE + int8 quant of the new token, cache patch,
layout shuffles); device does the masked attention sweep.

Layout/algorithm choices:
  * K/V caches shipped to HBM as bf16 (cast on host). The DMA engines
    charge the max(src,dst) side, so an int8->bf16 cast DMA costs the
    same as a raw bf16 copy -- but raw copies ride the hardware DGE
    (no ~1us/DMA SWDGE desc-gen on GpSimd, no queue DRAIN stalls), so
    exact per-batch loads with no group padding become free.
  * K stored pre-transposed [B, D, CACHE] so each chunk arrives as
    KT [d, s] and feeds matmul directly -- no PE transpose.
  * V stored position-interleaved [B, 16, 128, 2, D] so SBUF partition
    p holds positions {g*256 + j*128 + p}: per-partition contiguous
    runs are 512B (full DMA bandwidth) while the SBUF tile keeps
    partition = position-within-chunk.
  * One K DMA (sync HWDGE queue) + one V DMA (scalar HWDGE queue) per
    batch, sized exactly to that batch's chunk count; 4-deep tile
    buffering keeps the DMA engines saturated across batch boundaries.
  * Scores for G=8 chunks accumulate into one PSUM tile; the k_scaler
    multiply is a single DVE tensor_tensor against a stride-0-broadcast
    [128, G, 1->4] AP, then ONE batched ACT exp per G chunks (the ACT
    per-instruction PSUM-access overhead was the v2 bottleneck).
  * Mask via scalers only: masked positions have k_scaler = 0 (so
    exp(0) = 1 exactly) and v_scaler = 0 (no numerator contribution);
    the denominator over-count is the masked-position count, which the
    host subtracts exactly. No mask tensor, no -inf bias.
  * Denominator = sum_s exp accumulated as a per-partition vector on
    DVE (reduce over chunks + add); host does the final 128-partition
    sum. No per-chunk PE matmul for the denominator.
  * Numerator ov[d, r] += V_chunk.T @ pexp accumulates in PSUM across
    chunks; numerator and denominator ship out unnormalized, host
    divides.
  * scores->exp->PV chain software-pipelined at group granularity so
    the PE never head-of-line blocks on ACT/DVE.
"""

import os

os.environ.setdefault("JAX_PLATFORMS", "cpu")

import math
import numpy as np

B, H, KVH, D, CACHE = 16, 32, 8, 128, 4096
NREP = H // KVH
NCORES = 8
CHUNK = 128
VG = 2  # V chunks interleaved per 512B-run group (bf16)
NBG = 4  # batches per DMA group
G = int(os.environ.get("KERNEL_G", "8"))  # chunks per ACT/exp batch

_BUILD_CACHE = {}
LAST_RESULTS = None


def _rope(x, cos, sin):
    # x: [B, 1, Hx, D]; cos/sin: [B, 1, D//2]
    c = cos[:, :, None, :]
    s = sin[:, :, None, :]
    xe, xo = x[..., ::2], x[..., 1::2]
    re = xe * c - xo * s
    im = xe * s + xo * c
    return np.stack([re, im], axis=-1).reshape(x.shape).astype(np.float32)


def _build_program(ncs):
    """ncs: per-device-batch chunk counts, sorted ascending (same for
    every core since sharding is by kv head)."""
    from contextlib import ExitStack

    import concourse.bacc as bacc
    import concourse.tile as tile
    from concourse import mybir
    from concourse.bass import AP

    nc = bacc.Bacc()
    f32 = mybir.dt.float32
    bf16 = mybir.dt.bfloat16
    mult = mybir.AluOpType.mult
    add = mybir.AluOpType.add

    kt16 = nc.dram_tensor("kt16", [B, D, CACHE], bf16, kind="ExternalInput")
    v16 = nc.dram_tensor(
        "v16", [B, CACHE // (VG * CHUNK), CHUNK, VG, D], bf16, kind="ExternalInput"
    )
    qt = nc.dram_tensor("qt", [CHUNK, B, NREP], bf16, kind="ExternalInput")
    ksc = nc.dram_tensor("ksc", [CHUNK, B, CACHE // CHUNK], f32, kind="ExternalInput")
    vsc = nc.dram_tensor("vsc", [CHUNK, B, CACHE // CHUNK], f32, kind="ExternalInput")
    o_num = nc.dram_tensor("o_num", [CHUNK, B, NREP], f32, kind="ExternalOutput")
    o_den = nc.dram_tensor("o_den", [CHUNK, B, NREP], f32, kind="ExternalOutput")

    def bc(ap, n):
        # append a stride-0 axis of size n (free-axis broadcast)
        return AP(ap.tensor, ap.offset, list(ap.ap) + [[0, n]])

    with tile.TileContext(nc) as tc:
        with ExitStack() as ctx:
            singles = ctx.enter_context(tc.tile_pool(name="singles", bufs=1))
            sup = ctx.enter_context(tc.tile_pool(name="sup", bufs=4))
            sc_pool = ctx.enter_context(tc.tile_pool(name="sc", bufs=3, space="PSUM"))
            ov_pool = ctx.enter_context(tc.tile_pool(name="ov", bufs=2, space="PSUM"))
            sm_pool = ctx.enter_context(tc.tile_pool(name="sm", bufs=3))

            qt_all = singles.tile([CHUNK, B, NREP], bf16, tag="qta")
            nc.sync.dma_start(out=qt_all, in_=qt[:, :, :])
            ksc_all = singles.tile([CHUNK, B, CACHE // CHUNK], f32, tag="ksa")
            nc.scalar.dma_start(out=ksc_all, in_=ksc[:, :, :])
            vsc_all = singles.tile([CHUNK, B, CACHE // CHUNK], f32, tag="vsa")
            nc.scalar.dma_start(out=vsc_all, in_=vsc[:, :, :])
            out_num = singles.tile([CHUNK, B, NREP], f32, tag="onum")
            den_vec = singles.tile([CHUNK, B, NREP], f32, tag="oden")
            nc.vector.memset(den_vec, 0.0)

            for b in range(B):
                    nch = ncs[b]
                    ngg = -(-nch // VG)
                    ksrc = kt16[b, :, 0 : nch * CHUNK].rearrange(
                        "d (c s) -> d c s", s=CHUNK
                    )
                    ktsup = sup.tile([CHUNK, nch, CHUNK], bf16, tag="ksup")
                    nc.sync.dma_start(out=ktsup, in_=ksrc)
                    vsrc = v16[b, 0:ngg].rearrange("g p j d -> p g j d")
                    vsup = sup.tile([CHUNK, ngg, VG, CHUNK], bf16, tag="vsup")
                    nc.scalar.dma_start(out=vsup, in_=vsrc)
                    ngrp = -(-nch // G)
                    grps = [(k * G, min(G, nch - k * G)) for k in range(ngrp)]
                    ov = ov_pool.tile([CHUNK, NREP], f32)
                    pxs = [None] * ngrp

                    def front(k):
                        c0, gsz = grps[k]
                        scb = sc_pool.tile([CHUNK, G, NREP], f32)
                        for g in range(gsz):
                            nc.tensor.matmul(
                                scb[:, g, :],
                                lhsT=ktsup[:, c0 + g, :],
                                rhs=qt_all[:, b, :],
                                start=True,
                                stop=True,
                            )
                        sc_v = scb[:, 0:gsz, :]
                        nc.vector.tensor_tensor(
                            out=sc_v,
                            in0=sc_v,
                            in1=bc(ksc_all[:, b, c0 : c0 + gsz], NREP),
                            op=mult,
                        )
                        px = sm_pool.tile([CHUNK, G, NREP], bf16, tag="px")
                        px_v = px[:, 0:gsz, :]
                        nc.scalar.activation(
                            px_v, sc_v, mybir.ActivationFunctionType.Exp
                        )
                        tmp = sm_pool.tile([CHUNK, NREP], f32, tag="tmp")
                        nc.vector.tensor_reduce(
                            tmp,
                            px_v.rearrange("p g r -> p r g"),
                            axis=mybir.AxisListType.X,
                            op=add,
                        )
                        nc.vector.tensor_tensor(
                            out=den_vec[:, b, :],
                            in0=den_vec[:, b, :],
                            in1=tmp,
                            op=add,
                        )
                        nc.vector.tensor_tensor(
                            out=px_v,
                            in0=px_v,
                            in1=bc(vsc_all[:, b, c0 : c0 + gsz], NREP),
                            op=mult,
                        )
                        pxs[k] = px

                    def back(k):
                        c0, gsz = grps[k]
                        px = pxs[k]
                        for g in range(gsz):
                            c = c0 + g
                            nc.tensor.matmul(
                                ov,
                                lhsT=vsup[:, c // VG, c % VG, :],
                                rhs=px[:, g, :],
                                start=(c == 0),
                                stop=(c == nch - 1),
                            )
                        pxs[k] = None

                    front(0)
                    for k in range(1, ngrp):
                        front(k)
                        back(k - 1)
                    back(ngrp - 1)

                    nc.vector.tensor_copy(out_num[:, b, :], ov)

            nc.sync.dma_start(out=o_num[:, :, :], in_=out_num)
            nc.sync.dma_start(out=o_den[:, :, :], in_=den_vec)

    nc.compile()
    return nc


def kernel(
    xq,
    xk,
    xv,
    freqs_cos,
    freqs_sin,
    k_scaler,
    v_scaler,
    cache_k,
    cache_v,
    input_pos,
):
    global LAST_RESULTS
    import ml_dtypes
    from concourse.bass_utils import run_bass_kernel_spmd

    bf16 = ml_dtypes.bfloat16
    xq = np.asarray(xq, np.float32)
    xk = np.asarray(xk, np.float32)
    xv = np.asarray(xv, np.float32)
    freqs_cos = np.asarray(freqs_cos, np.float32)
    freqs_sin = np.asarray(freqs_sin, np.float32)
    k_scaler = np.asarray(k_scaler, np.float32)
    v_scaler = np.asarray(v_scaler, np.float32)
    cache_k = np.asarray(cache_k)
    cache_v = np.asarray(cache_v)
    input_pos = np.asarray(input_pos)
    pos = input_pos.astype(np.int64)

    # --- tiny host prep: RoPE + int8 quantization of the single new token ---
    q = _rope(xq, freqs_cos, freqs_sin)[:, 0]  # [B, H, D]
    k = _rope(xk, freqs_cos, freqs_sin)[:, 0]  # [B, KVH, D]
    v_new = xv[:, 0]  # [B, KVH, D]
    k_s = (np.max(np.abs(k), axis=-1, keepdims=True) / np.float32(127.0)).astype(
        np.float32
    ) + np.float32(1e-8)
    v_s = (np.max(np.abs(v_new), axis=-1, keepdims=True) / np.float32(127.0)).astype(
        np.float32
    ) + np.float32(1e-8)
    k_q = np.clip(np.round(k / k_s), -127, 127).astype(np.int8)
    v_q = np.clip(np.round(v_new / v_s), -127, 127).astype(np.int8)

    # device batch order: ascending chunk count (shrinks DMA-group padding)
    ncs_raw = pos // CHUNK + 1
    order = np.argsort(ncs_raw, kind="stable")
    ncs = tuple(int(ncs_raw[b]) for b in order)

    if ncs not in _BUILD_CACHE:
        _BUILD_CACHE[ncs] = _build_program(ncs)
    nc = _BUILD_CACHE[ncs]

    bidx = np.arange(B)
    inv_sqrt_d = np.float32(1.0 / math.sqrt(D))
    s_idx = np.arange(CACHE, dtype=np.int64)
    masked = s_idx[None, :] > pos[:, None]  # [B, CACHE] True -> excluded
    masked_dev = masked[order]
    pos_dev = pos[order]
    # masked positions inside processed chunks contribute exp(0)=1 each
    n_masked = (np.asarray(ncs, np.int64) * CHUNK - (pos_dev + 1)).astype(
        np.float32
    )  # [B] device order

    def chunk_layout(a):  # [B, CACHE] -> [128, B, 32] with s = c*128 + p
        return np.ascontiguousarray(
            a.reshape(B, CACHE // CHUNK, CHUNK).transpose(2, 0, 1)
        )

    in_maps = []
    for m in range(NCORES):
        ck_m = cache_k[:, m].astype(np.int8)  # [B, CACHE, D]
        cv_m = cache_v[:, m].astype(np.int8)
        ck_m[bidx, pos, :] = k_q[:, m]
        cv_m[bidx, pos, :] = v_q[:, m]
        ck_m = ck_m[order].astype(bf16)
        cv_m = cv_m[order].astype(bf16)

        kt16 = np.ascontiguousarray(ck_m.transpose(0, 2, 1))  # [B, D, CACHE]
        v16 = np.ascontiguousarray(
            cv_m.reshape(B, CACHE // (VG * CHUNK), VG, CHUNK, D).transpose(
                0, 1, 3, 2, 4
            )
        )  # [B, 8, 128, VG, D]

        ks_m = k_scaler[:, m].copy()  # [B, CACHE]
        vs_m = v_scaler[:, m].copy()
        ks_m[bidx, pos] = k_s[:, m, 0]
        vs_m[bidx, pos] = v_s[:, m, 0]
        ks_m = ks_m[order]
        vs_m = vs_m[order]

        ks_m *= inv_sqrt_d
        ks_m[masked_dev] = np.float32(0.0)
        vs_m[masked_dev] = np.float32(0.0)

        qt_m = np.ascontiguousarray(
            q[order][:, m * NREP : (m + 1) * NREP, :].transpose(2, 0, 1)
        ).astype(bf16)  # [D, B, NREP]

        in_maps.append(
            dict(
                kt16=kt16,
                v16=v16,
                qt=qt_m,
                ksc=chunk_layout(ks_m),
                vsc=chunk_layout(vs_m),
            )
        )

    res = run_bass_kernel_spmd(nc, in_maps, core_ids=list(range(NCORES)))
    LAST_RESULTS = res

    inv_order = np.empty(B, np.int64)
    inv_order[order] = np.arange(B)
    out = np.zeros((B, H, 1, D), np.float32)
    for m in range(NCORES):
        num = np.asarray(res.results[m]["o_num"], np.float32)  # [D, B, NREP]
        dvec = np.asarray(res.results[m]["o_den"], np.float32)  # [128, B, NREP]
        den = dvec.sum(axis=0) - n_masked[:, None]  # [B, NREP]
        o = (num / den[None, :, :]).transpose(1, 2, 0)  # [B, NREP, D]
        out[:, m * NREP : (m + 1) * NREP, 0, :] = o[inv_order]
    return out
